# revision 4
# baseline (speedup 1.0000x reference)
"""Trainium2 Bass kernel v9: DifferentOptionsPolicyNetwork, bf16 datapath.

Changes vs v5 (15.06us):
  - Packed single output: stage-3 runs as TWO matmuls ([Wm|bm] and
    [Ws|bs]) into a 2-bank PSUM pair [32, 1024] (mean in bank A cols
    0:280, log_std in bank B cols 512:792); the PSUM->SBUF move runs
    as two PARALLEL flat ops (scalar ACT-Copy for mean, vector
    min/max clip for log_std — clipping mean is a no-op anyway since
    |out| <= 0.25) finishing ~t3+510, well before the output DMA
    engines read out_sb; ONE output DMA moves the contiguous [32,560]
    bf16 block as 32 wide 1120-byte packets.
  - The output DMA is issued by the otherwise-idle GpSimd SWDGE gated
    on the h2x relu (sa>=3): its ~1.7us gen+delay covers stage-3 +
    clip with ~320ns margin even at the cold 1.2GHz PE clock.
  - The output DMA keeps a completion semaphore (walrus requires at
    least one sync update on DMA instructions) but nothing waits on it
    (NRT epilogue grace covers the write, as in the baseline).
"""

import sys
import types

import numpy as np

B, I, O, H, A = 2048, 256, 8, 512, 32
K = H // O
NPAD = 280
N_CORES = 8
LOG_STD_MIN, LOG_STD_MAX = -20.0, 2.0
W_WARM = 21

C_Z = 0
C_X0 = 1
C_W10 = C_X0 + NPAD          # 281
C_X1 = C_W10 + H             # 793
C_W11 = C_X1 + NPAD          # 1073
C_W2 = C_W11 + H             # 1585
C_SM = C_W2 + 4 * K          # 1841
C_TOT = C_SM + 2 * A         # 1905


def _ensure_axon_hooks_shim():
    try:
        import antenv.axon_hooks  # noqa: F401
        return
    except ImportError:
        pass
    try:
        import antenv
    except ImportError:
        return
    mod = types.ModuleType("antenv.axon_hooks")
    mod._hook = None
    mod.set_axon_ntff_profile_hook = lambda h: setattr(mod, "_hook", h)
    mod.get_axon_ntff_profile_hook = lambda: mod._hook
    sys.modules["antenv.axon_hooks"] = mod
    antenv.axon_hooks = mod


_cached_nc = None
last_run = None


def _build_nc():
    import concourse.bass as bass
    import concourse.mybir as mybir
    from concourse import bacc

    f32 = mybir.dt.float32
    bf16 = mybir.dt.bfloat16
    Act = mybir.ActivationFunctionType
    Alu = mybir.AluOpType

    patches = []
    try:
        patches.append((bass.Bass, "all_engine_barrier", bass.Bass.all_engine_barrier))
        bass.Bass.all_engine_barrier = lambda self, *, sem_only=False: None
    except Exception:
        pass
    try:
        patches.append((bass.BassGpSimd, "memset", None))
        bass.BassGpSimd.memset = lambda self, ap, constant: None
    except Exception:
        patches.pop()
    try:
        nc = bacc.Bacc("TRN2", target_bir_lowering=False, debug=False)
    finally:
        for cls, name, orig in reversed(patches):
            if orig is None:
                delattr(cls, name)
            else:
                setattr(cls, name, orig)

    pack = nc.dram_tensor("pack", [128, C_TOT], bf16, kind="ExternalInput")
    outT = nc.dram_tensor("outT", [A, 2 * NPAD], bf16, kind="ExternalOutput")

    ctx_mgrs = []

    def alloc(cm):
        ctx_mgrs.append(cm)
        return cm.__enter__()

    try:
        pk = alloc(nc.sbuf_tensor("pk", [128, C_TOT], bf16))
        h1_sb = alloc(nc.sbuf_tensor("h1s", [128, 4 * NPAD], bf16))
        h2x_sb = alloc(nc.sbuf_tensor("h2xs", [K + 1, NPAD], bf16))
        out_sb = alloc(nc.sbuf_tensor("outs", [A, 2 * NPAD], bf16))
        warm_sb = alloc(nc.sbuf_tensor("warms", [128, 128], bf16))

        h1_ps = [alloc(nc.psum_tensor(f"h1p{i}", [128, NPAD], f32)) for i in range(4)]
        h2_ps = alloc(nc.psum_tensor("h2p", [K, NPAD], f32))
        out_ps = alloc(nc.psum_tensor("outp", [A, 1024], f32))
        warm_ps = alloc(nc.psum_tensor("warmp", [128, 128], f32))

        p1 = alloc(nc.semaphore("p1"))
        p2 = alloc(nc.semaphore("p2"))
        p3 = alloc(nc.semaphore("p3"))
        p4 = alloc(nc.semaphore("p4"))
        gw = alloc(nc.semaphore("gw"))
        sem_e = alloc(nc.semaphore("sem_e"))
        t1 = alloc(nc.semaphore("t1"))
        t2 = alloc(nc.semaphore("t2"))
        t3 = alloc(nc.semaphore("t3"))
        sa = alloc(nc.semaphore("sa"))
        sv = alloc(nc.semaphore("sv"))
        svw = alloc(nc.semaphore("svw"))

        x0 = pk.ap()[:, C_X0 : C_X0 + NPAD]
        x1 = pk.ap()[:, C_X1 : C_X1 + NPAD]
        w10 = pk.ap()[:, C_W10 : C_W10 + H]
        w11 = pk.ap()[:, C_W11 : C_W11 + H]
        w2 = pk.ap()[:, C_W2 : C_W2 + 4 * K]
        smm = pk.ap()[0 : K + 1, C_SM : C_SM + A]
        sms = pk.ap()[0 : K + 1, C_SM + A : C_SM + 2 * A]
        zb128 = pk.ap()[0:128, C_Z : C_Z + 1]
        zb64 = pk.ap()[0:K, C_Z : C_Z + 1]

        def dma_in(eng, lo, hi, sem):
            eng.dma_start(pk.ap()[:, lo:hi], pack.ap()[:, lo:hi]).then_inc(sem, 16)

        scalar = nc.scalar
        if True:
            dma_in(scalar, 0, C_W10 + 128, p1)            # z | x0 | w10 t0
            dma_in(scalar, C_W11, C_W2, p3)               # w11 (whole)
            scalar.wait_ge(t1, 1)
            scalar.activation(
                h1_sb[:, 0 * NPAD : 1 * NPAD], h1_ps[0][:], Act.Relu, bias=zb128
            ).then_inc(sa, 1)
            scalar.wait_ge(t1, 3)
            scalar.activation(
                h1_sb[:, 2 * NPAD : 3 * NPAD], h1_ps[2][:], Act.Relu, bias=zb128
            ).then_inc(sa, 1)
            scalar.wait_ge(t2, 1)
            scalar.activation(h2x_sb[0:K, :], h2_ps[:], Act.Relu, bias=zb64).then_inc(sa, 1)
            scalar.wait_ge(t3, 1)
            scalar.activation(out_sb[:, 0:NPAD], out_ps.ap()[:, 0:NPAD], Act.Copy)


        sync = nc.sync
        if True:
            dma_in(sync, C_W10 + 128, C_X1, p2)           # w10 t1,t2,t3
            dma_in(sync, C_X1, C_W11, p4)                 # x1

        gpsimd = nc.gpsimd
        if True:
            dma_in(gpsimd, C_W2, C_TOT, gw)               # w2 | sm
            gpsimd.wait_ge(sa, 3)
            gpsimd.dma_start(outT[:], out_sb[:, :]).then_inc(sem_e, 16)

        vector = nc.vector
        if True:
            vector.memset(h2x_sb[K : K + 1, :], 1.0).then_inc(svw, 1)
            vector.wait_ge(t1, 2)
            vector.tensor_scalar_max(
                h1_sb[:, 1 * NPAD : 2 * NPAD], h1_ps[1][:], 0.0
            ).then_inc(sv, 1)
            vector.wait_ge(t1, 4)
            vector.tensor_scalar_max(
                h1_sb[:, 3 * NPAD : 4 * NPAD], h1_ps[3][:], 0.0
            ).then_inc(sv, 1)
            vector.wait_ge(t3, 1)
            vector.tensor_scalar(
                out=out_sb[:, NPAD : 2 * NPAD],
                in0=out_ps.ap()[:, 512 : 512 + NPAD],
                scalar1=LOG_STD_MAX,
                scalar2=LOG_STD_MIN,
                op0=Alu.min,
                op1=Alu.max,
            ).then_inc(sv, 1)

        tensor = nc.tensor
        if True:
            for _ in range(W_WARM):
                tensor.matmul(
                    warm_ps[:], warm_sb[:], warm_sb[:], start=True, stop=True
                )
            # stage 1, contraction chunk 0 (x0 @ w10), all four h tiles
            tensor.wait_ge(p2, 16)
            tensor.wait_ge(p1, 16)
            for h in range(4):
                tensor.matmul(
                    h1_ps[h][:],
                    w10[:, h * 128 : (h + 1) * 128],
                    x0,
                    start=True,
                    stop=False,
                    skip_group_check=True,
                )
            # stage 1, contraction chunk 1 (x1 @ w11)
            tensor.wait_ge(p4, 16)
            tensor.wait_ge(p3, 16)
            for h in range(4):
                tensor.matmul(
                    h1_ps[h][:],
                    w11[:, h * 128 : (h + 1) * 128],
                    x1,
                    start=False,
                    stop=True,
                    skip_group_check=True,
                ).then_inc(t1, 1)
            # stage 2
            tensor.wait_ge(gw, 16)
            stage2_waits = [(sa, 1), (sv, 1), (sa, 2), (sv, 2)]
            for c in range(4):
                sem, val = stage2_waits[c]
                tensor.wait_ge(sem, val)
                mm = tensor.matmul(
                    h2_ps[:],
                    w2[:, c * K : (c + 1) * K],
                    h1_sb[:, c * NPAD : (c + 1) * NPAD],
                    start=(c == 0),
                    stop=(c == 3),
                )
            mm.then_inc(t2, 1)
            # stage 3: mean into bank A cols 0:NPAD, log_std into bank B cols 512:512+NPAD
            tensor.wait_ge(sa, 3)
            tensor.wait_ge(svw, 1)
            tensor.matmul(
                out_ps.ap()[:, 0:NPAD], smm, h2x_sb[:], start=True, stop=True
            )
            tensor.matmul(
                out_ps.ap()[:, 512 : 512 + NPAD], sms, h2x_sb[:], start=True, stop=True
            ).then_inc(t3, 1)

    finally:
        for cm in reversed(ctx_mgrs):
            cm.__exit__(None, None, None)

    nc.compile()
    return nc


def _numpy_fallback(state, W1, W2, Wm, Ws, bm, bs, opt):
    x = np.maximum(np.einsum("bi,bih->bh", state, W1[opt]), 0.0)
    x = np.maximum(np.einsum("bh,bhk->bk", x, W2[opt]), 0.0)
    mean = np.einsum("bk,bka->ba", x, Wm[opt]) + bm[opt]
    ls = np.einsum("bk,bka->ba", x, Ws[opt]) + bs[opt]
    return mean.astype(np.float32), np.clip(ls, LOG_STD_MIN, LOG_STD_MAX).astype(
        np.float32
    )


def kernel(state, W1, W2, Wm, Ws, bm, bs, option):
    global _cached_nc, last_run
    _ensure_axon_hooks_shim()
    import ml_dtypes
    from concourse.bass_utils import run_bass_kernel_spmd

    bft = ml_dtypes.bfloat16
    state = np.ascontiguousarray(np.asarray(state, dtype=np.float32))
    W1 = np.asarray(W1, dtype=np.float32)
    W2 = np.asarray(W2, dtype=np.float32)
    Wm = np.asarray(Wm, dtype=np.float32)
    Ws = np.asarray(Ws, dtype=np.float32)
    bm = np.asarray(bm, dtype=np.float32)
    bs = np.asarray(bs, dtype=np.float32)
    opt = np.asarray(option).astype(np.int32)

    idx = [np.nonzero(opt == o)[0] for o in range(O)]
    if max(len(ix) for ix in idx) > NPAD:
        return _numpy_fallback(state, W1, W2, Wm, Ws, bm, bs, opt)

    in_maps = []
    for o in range(O):
        ix = idx[o]
        pk = np.zeros((128, C_TOT), bft)
        xT = state[ix].T.astype(bft)  # [256, n]
        pk[:, C_X0 : C_X0 + len(ix)] = xT[0:128]
        pk[:, C_X1 : C_X1 + len(ix)] = xT[128:256]
        pk[:, C_W10 : C_W10 + H] = W1[o][0:128].astype(bft)
        pk[:, C_W11 : C_W11 + H] = W1[o][128:256].astype(bft)
        pk[:, C_W2 : C_W2 + 4 * K] = (
            W2[o].reshape(4, 128, K).transpose(1, 0, 2).reshape(128, 4 * K).astype(bft)
        )
        pk[0:K, C_SM : C_SM + A] = Wm[o].astype(bft)
        pk[0:K, C_SM + A : C_SM + 2 * A] = Ws[o].astype(bft)
        pk[K, C_SM : C_SM + A] = bm[o].astype(bft)
        pk[K, C_SM + A : C_SM + 2 * A] = bs[o].astype(bft)
        in_maps.append({"pack": pk})

    if _cached_nc is None:
        _cached_nc = _build_nc()

    last_run = run_bass_kernel_spmd(_cached_nc, in_maps, core_ids=list(range(N_CORES)))

    mean = np.empty((B, A), np.float32)
    log_std = np.empty((B, A), np.float32)
    for o in range(O):
        ix = idx[o]
        res = last_run.results[o]["outT"]
        mean[ix] = res[:, : len(ix)].T.astype(np.float32)
        log_std[ix] = res[:, NPAD : NPAD + len(ix)].T.astype(np.float32)
    return mean, log_std


# revision 5
# speedup vs baseline: 1.0612x; 1.0612x over previous
"""Trainium2 Bass kernel: DifferentOptionsPolicyNetwork (MoE option routing).

Expert-parallel across 8 NeuronCores: samples are grouped by option on
the host (free), core o gets option o's weights + its samples
transposed and zero-padded to NPAD=280 columns (fixed-seed max option
count is 275; larger counts take the exact numpy fallback), so every
device matmul is dense. Whole datapath is bf16 with fp32 PSUM
accumulation (end-to-end rel err ~5e-3 vs the 2e-2 gate); bf16
matmuls are single-pass where fp32 was a LOW/HIGH double-pass.

Pipeline per core (raw Bacc, flat emission, manual semaphores; init
all-engine barrier and const-AP memsets patched out):
  - Input pack [128, 1905] bf16 split into 5 pieces over 3 DMA queues,
    wide rows only (narrow pieces are packet-overhead-bound; aggregate
    DMA caps ~200GB/s): Scalar HWDGE [z|x0|w10t0] then [w11]; Sync
    HWDGE [w10t123] then [x1]; GpSimd SWDGE [w2|sm]. Piece-level
    semaphores gate stage-1 so chunk 0 starts as soon as x0+w10 land
    (~8.6us) and chunk 1 is fed exactly as chunk 0 retires.
  - 21 bf16 warmup matmuls keep the PE queue primed through the DMA-in
    window (ending before the first piece lands).
  - stage1: h1[512,n] = relu(W1.T @ x), 2 contraction chunks x 4
    h-tiles; ReLUs split scalar(h0,h2)/vector(h1,h3) writing bf16;
    stage2: h2[64,n] = relu(W2c.T @ h1) accumulated over 4 h-chunks,
    gated per-tile on the relus; stage3: TWO matmuls [Wm|bm] and
    [Ws|bs] (bias via a ones row memset early on vector) into a 2-bank
    PSUM pair [32, 1024] (mean cols 0:280, log_std cols 512:792).
  - PSUM->SBUF: two PARALLEL flat ops gated on t3 (scalar ACT-Copy for
    mean, vector min/max clip for log_std) into a contiguous [32,560]
    bf16 block, done ~t3+510.
  - ONE output DMA [32 x 1120B] issued by the otherwise-idle GpSimd
    SWDGE gated on the h2x relu (sa>=3): its ~1.6us wake+gen+delay
    overlaps stage-3 and the copy/clip with stable margin. Nothing
    waits on its completion semaphore (walrus requires one on DMA
    instructions); the NRT epilogue grace window covers the write.

Measured on this env: ~14.4-15.6us vs 20.5us fp32 baseline.
"""

import sys
import types

import numpy as np

B, I, O, H, A = 2048, 256, 8, 512, 32
K = H // O
NPAD = 280
N_CORES = 8
LOG_STD_MIN, LOG_STD_MAX = -20.0, 2.0
W_WARM = 21

C_Z = 0
C_X0 = 1
C_W10 = C_X0 + NPAD          # 281
C_X1 = C_W10 + H             # 793
C_W11 = C_X1 + NPAD          # 1073
C_W2 = C_W11 + H             # 1585
C_SM = C_W2 + 4 * K          # 1841
C_TOT = C_SM + 2 * A         # 1905


def _ensure_axon_hooks_shim():
    try:
        import antenv.axon_hooks  # noqa: F401
        return
    except ImportError:
        pass
    try:
        import antenv
    except ImportError:
        return
    mod = types.ModuleType("antenv.axon_hooks")
    mod._hook = None
    mod.set_axon_ntff_profile_hook = lambda h: setattr(mod, "_hook", h)
    mod.get_axon_ntff_profile_hook = lambda: mod._hook
    sys.modules["antenv.axon_hooks"] = mod
    antenv.axon_hooks = mod


_cached_nc = None
last_run = None


def _build_nc():
    import concourse.bass as bass
    import concourse.mybir as mybir
    from concourse import bacc

    f32 = mybir.dt.float32
    bf16 = mybir.dt.bfloat16
    Act = mybir.ActivationFunctionType
    Alu = mybir.AluOpType

    patches = []
    try:
        patches.append((bass.Bass, "all_engine_barrier", bass.Bass.all_engine_barrier))
        bass.Bass.all_engine_barrier = lambda self, *, sem_only=False: None
    except Exception:
        pass
    try:
        patches.append((bass.BassGpSimd, "memset", None))
        bass.BassGpSimd.memset = lambda self, ap, constant: None
    except Exception:
        patches.pop()
    try:
        nc = bacc.Bacc("TRN2", target_bir_lowering=False, debug=False)
    finally:
        for cls, name, orig in reversed(patches):
            if orig is None:
                delattr(cls, name)
            else:
                setattr(cls, name, orig)

    pack = nc.dram_tensor("pack", [128, C_TOT], bf16, kind="ExternalInput")
    outT = nc.dram_tensor("outT", [A, 2 * NPAD], bf16, kind="ExternalOutput")

    ctx_mgrs = []

    def alloc(cm):
        ctx_mgrs.append(cm)
        return cm.__enter__()

    try:
        pk = alloc(nc.sbuf_tensor("pk", [128, C_TOT], bf16))
        h1_sb = alloc(nc.sbuf_tensor("h1s", [128, 4 * NPAD], bf16))
        h2x_sb = alloc(nc.sbuf_tensor("h2xs", [K + 1, NPAD], bf16))
        out_sb = alloc(nc.sbuf_tensor("outs", [A, 2 * NPAD], bf16))
        warm_sb = alloc(nc.sbuf_tensor("warms", [128, 128], bf16))

        h1_ps = [alloc(nc.psum_tensor(f"h1p{i}", [128, NPAD], f32)) for i in range(4)]
        h2_ps = alloc(nc.psum_tensor("h2p", [K, NPAD], f32))
        out_ps = alloc(nc.psum_tensor("outp", [A, 1024], f32))
        warm_ps = alloc(nc.psum_tensor("warmp", [128, 128], f32))

        p1 = alloc(nc.semaphore("p1"))
        p2 = alloc(nc.semaphore("p2"))
        p3 = alloc(nc.semaphore("p3"))
        p4 = alloc(nc.semaphore("p4"))
        gw = alloc(nc.semaphore("gw"))
        sem_e = alloc(nc.semaphore("sem_e"))
        t1 = alloc(nc.semaphore("t1"))
        t2 = alloc(nc.semaphore("t2"))
        t3 = alloc(nc.semaphore("t3"))
        sa = alloc(nc.semaphore("sa"))
        sv = alloc(nc.semaphore("sv"))
        svw = alloc(nc.semaphore("svw"))

        x0 = pk.ap()[:, C_X0 : C_X0 + NPAD]
        x1 = pk.ap()[:, C_X1 : C_X1 + NPAD]
        w10 = pk.ap()[:, C_W10 : C_W10 + H]
        w11 = pk.ap()[:, C_W11 : C_W11 + H]
        w2 = pk.ap()[:, C_W2 : C_W2 + 4 * K]
        smm = pk.ap()[0 : K + 1, C_SM : C_SM + A]
        sms = pk.ap()[0 : K + 1, C_SM + A : C_SM + 2 * A]
        zb128 = pk.ap()[0:128, C_Z : C_Z + 1]
        zb64 = pk.ap()[0:K, C_Z : C_Z + 1]

        def dma_in(eng, lo, hi, sem):
            eng.dma_start(pk.ap()[:, lo:hi], pack.ap()[:, lo:hi]).then_inc(sem, 16)

        scalar = nc.scalar
        if True:
            dma_in(scalar, 0, C_W10 + 128, p1)            # z | x0 | w10 t0
            dma_in(scalar, C_W11, C_W2, p3)               # w11 (whole)
            scalar.wait_ge(t1, 1)
            scalar.activation(
                h1_sb[:, 0 * NPAD : 1 * NPAD], h1_ps[0][:], Act.Relu, bias=zb128
            ).then_inc(sa, 1)
            scalar.wait_ge(t1, 3)
            scalar.activation(
                h1_sb[:, 2 * NPAD : 3 * NPAD], h1_ps[2][:], Act.Relu, bias=zb128
            ).then_inc(sa, 1)
            scalar.wait_ge(t2, 1)
            scalar.activation(h2x_sb[0:K, :], h2_ps[:], Act.Relu, bias=zb64).then_inc(sa, 1)
            scalar.wait_ge(t3, 1)
            scalar.activation(out_sb[:, 0:NPAD], out_ps.ap()[:, 0:NPAD], Act.Copy)


        sync = nc.sync
        if True:
            dma_in(sync, C_W10 + 128, C_X1, p2)           # w10 t1,t2,t3
            dma_in(sync, C_X1, C_W11, p4)                 # x1

        gpsimd = nc.gpsimd
        if True:
            dma_in(gpsimd, C_W2, C_TOT, gw)               # w2 | sm
            gpsimd.wait_ge(sa, 3)
            gpsimd.dma_start(outT[:], out_sb[:, :]).then_inc(sem_e, 16)

        vector = nc.vector
        if True:
            vector.memset(h2x_sb[K : K + 1, :], 1.0).then_inc(svw, 1)
            vector.wait_ge(t1, 2)
            vector.tensor_scalar_max(
                h1_sb[:, 1 * NPAD : 2 * NPAD], h1_ps[1][:], 0.0
            ).then_inc(sv, 1)
            vector.wait_ge(t1, 4)
            vector.tensor_scalar_max(
                h1_sb[:, 3 * NPAD : 4 * NPAD], h1_ps[3][:], 0.0
            ).then_inc(sv, 1)
            vector.wait_ge(t3, 1)
            vector.tensor_scalar(
                out=out_sb[:, NPAD : 2 * NPAD],
                in0=out_ps.ap()[:, 512 : 512 + NPAD],
                scalar1=LOG_STD_MAX,
                scalar2=LOG_STD_MIN,
                op0=Alu.min,
                op1=Alu.max,
            ).then_inc(sv, 1)

        tensor = nc.tensor
        if True:
            for _ in range(W_WARM):
                tensor.matmul(
                    warm_ps[:], warm_sb[:], warm_sb[:], start=True, stop=True
                )
            # stage 1, contraction chunk 0 (x0 @ w10), all four h tiles
            tensor.wait_ge(p2, 16)
            tensor.wait_ge(p1, 16)
            for h in range(4):
                tensor.matmul(
                    h1_ps[h][:],
                    w10[:, h * 128 : (h + 1) * 128],
                    x0,
                    start=True,
                    stop=False,
                    skip_group_check=True,
                )
            # stage 1, contraction chunk 1 (x1 @ w11)
            tensor.wait_ge(p4, 16)
            tensor.wait_ge(p3, 16)
            for h in range(4):
                tensor.matmul(
                    h1_ps[h][:],
                    w11[:, h * 128 : (h + 1) * 128],
                    x1,
                    start=False,
                    stop=True,
                    skip_group_check=True,
                ).then_inc(t1, 1)
            # stage 2
            tensor.wait_ge(gw, 16)
            stage2_waits = [(sa, 1), (sv, 1), (sa, 2), (sv, 2)]
            for c in range(4):
                sem, val = stage2_waits[c]
                tensor.wait_ge(sem, val)
                mm = tensor.matmul(
                    h2_ps[:],
                    w2[:, c * K : (c + 1) * K],
                    h1_sb[:, c * NPAD : (c + 1) * NPAD],
                    start=(c == 0),
                    stop=(c == 3),
                )
            mm.then_inc(t2, 1)
            # stage 3: mean into bank A cols 0:NPAD, log_std into bank B cols 512:512+NPAD
            tensor.wait_ge(sa, 3)
            tensor.wait_ge(svw, 1)
            tensor.matmul(
                out_ps.ap()[:, 0:NPAD], smm, h2x_sb[:], start=True, stop=True
            )
            tensor.matmul(
                out_ps.ap()[:, 512 : 512 + NPAD], sms, h2x_sb[:], start=True, stop=True
            ).then_inc(t3, 1)

    finally:
        for cm in reversed(ctx_mgrs):
            cm.__exit__(None, None, None)

    nc.compile()
    return nc


def _numpy_fallback(state, W1, W2, Wm, Ws, bm, bs, opt):
    x = np.maximum(np.einsum("bi,bih->bh", state, W1[opt]), 0.0)
    x = np.maximum(np.einsum("bh,bhk->bk", x, W2[opt]), 0.0)
    mean = np.einsum("bk,bka->ba", x, Wm[opt]) + bm[opt]
    ls = np.einsum("bk,bka->ba", x, Ws[opt]) + bs[opt]
    return mean.astype(np.float32), np.clip(ls, LOG_STD_MIN, LOG_STD_MAX).astype(
        np.float32
    )


def kernel(state, W1, W2, Wm, Ws, bm, bs, option):
    global _cached_nc, last_run
    _ensure_axon_hooks_shim()
    import ml_dtypes
    from concourse.bass_utils import run_bass_kernel_spmd

    bft = ml_dtypes.bfloat16
    state = np.ascontiguousarray(np.asarray(state, dtype=np.float32))
    W1 = np.asarray(W1, dtype=np.float32)
    W2 = np.asarray(W2, dtype=np.float32)
    Wm = np.asarray(Wm, dtype=np.float32)
    Ws = np.asarray(Ws, dtype=np.float32)
    bm = np.asarray(bm, dtype=np.float32)
    bs = np.asarray(bs, dtype=np.float32)
    opt = np.asarray(option).astype(np.int32)

    idx = [np.nonzero(opt == o)[0] for o in range(O)]
    if max(len(ix) for ix in idx) > NPAD:
        return _numpy_fallback(state, W1, W2, Wm, Ws, bm, bs, opt)

    in_maps = []
    for o in range(O):
        ix = idx[o]
        pk = np.zeros((128, C_TOT), bft)
        xT = state[ix].T.astype(bft)  # [256, n]
        pk[:, C_X0 : C_X0 + len(ix)] = xT[0:128]
        pk[:, C_X1 : C_X1 + len(ix)] = xT[128:256]
        pk[:, C_W10 : C_W10 + H] = W1[o][0:128].astype(bft)
        pk[:, C_W11 : C_W11 + H] = W1[o][128:256].astype(bft)
        pk[:, C_W2 : C_W2 + 4 * K] = (
            W2[o].reshape(4, 128, K).transpose(1, 0, 2).reshape(128, 4 * K).astype(bft)
        )
        pk[0:K, C_SM : C_SM + A] = Wm[o].astype(bft)
        pk[0:K, C_SM + A : C_SM + 2 * A] = Ws[o].astype(bft)
        pk[K, C_SM : C_SM + A] = bm[o].astype(bft)
        pk[K, C_SM + A : C_SM + 2 * A] = bs[o].astype(bft)
        in_maps.append({"pack": pk})

    if _cached_nc is None:
        _cached_nc = _build_nc()

    last_run = run_bass_kernel_spmd(_cached_nc, in_maps, core_ids=list(range(N_CORES)))

    mean = np.empty((B, A), np.float32)
    log_std = np.empty((B, A), np.float32)
    for o in range(O):
        ix = idx[o]
        res = last_run.results[o]["outT"]
        mean[ix] = res[:, : len(ix)].T.astype(np.float32)
        log_std[ix] = res[:, NPAD : NPAD + len(ix)].T.astype(np.float32)
    return mean, log_std
